# revision 44
# baseline (speedup 1.0000x reference)
"""Trainium2 Bass kernel: CustomPatchEmbedding.

gather 16x16x3 patches at runtime (h_idx, w_idx) + 768x768 linear projection.

kernel(**inputs) takes FULL unsharded inputs
  x [32,3,384,384] f32, h_idx/w_idx [32,576] i32, proj_w [768,768] f32,
  proj_b [768] f32  ->  out [32,576,768] f32.

Sharding: data-parallel batch across 8 NeuronCores (4 images each).

Device-side gather primitive on this toolchain: SWDGE indirect DMA with ONE
dynamic offset per partition, streaming the dest free dim contiguously from
that offset. To make each gathered run long, the host re-packs x into a
quad-row-interleaved HWC layout
    Q[b][q][w][c][r] = x[b, c, 4q + r, w]
so one run = 16 pixels x 12 (c,r) = 192 f32 = 768B covers FOUR patch rows
of all channels. A patch (h = 4*q0 + s) needs quads q0..q0+4: 5 runs.

The 960 gathered columns interleave useful rows with garbage rows (which
rows depends on s = h%4), so patches are bucketed by s (4 buckets x 6
chunks of 128, padded with dummy slots) and the matmul contracts K=1024
against host-built zero-padded weight variants W5[s], which zero out the
garbage columns. Outputs are scattered back to original patch positions
via per-partition indirect scatter.
"""

import os

import numpy as np

USE_BF16 = os.environ.get("KERNEL_MM_F32", "") != "1"  # bf16 matmul default

PH, PW = 16, 16
EMBED = 768
B, C, H, W = 32, 3, 384, 384
N = 576
NCORES = 8
BPC = B // NCORES            # images per core
M = BPC * N                  # real patches per core (2304)
NQ = H // 4                  # quads per image column (96)
V = BPC * C * H * W          # elements in the core's Q slice
NB = 4                       # s buckets
CPB = 5                      # chunks per bucket
NCHUNK = NB * CPB            # 24
SLOTS = NCHUNK * 128         # 3072 slots
KPAD = 1024                  # contract dim (960 gathered + 64 pad)
RUNL = 192                   # elements per gather run (16 px * 12)
OUTROWS = SLOTS              # rows come back in slot order

_cache = {}


def _emit_body(nc, tc, bass, mybir, aps, n_chunks, cpb, reps=1):
    dt = mybir.dt
    q_d, offs_d, w_d, bias_d, out_d = (
        aps["q"], aps["offs"], aps["w5"], aps["bias"], aps["out"])

    gdt = dt.float32
    mdt = dt.bfloat16 if USE_BF16 else dt.float32
    with tc.tile_pool(name="const", bufs=1) as cpool, \
         tc.tile_pool(name="gath", bufs=4) as gpool, \
         tc.tile_pool(name="work", bufs=3) as wpool, \
         tc.tile_pool(name="psumt", bufs=2, space="PSUM") as tpool, \
         tc.tile_pool(name="psuma", bufs=3, space="PSUM") as apool, \
         tc.tile_pool(name="outp", bufs=3) as opool:
        from concourse.masks import make_identity
        ident = cpool.tile([128, 128], gdt)
        make_identity(nc, ident[:])
        # W5 variants: [4 s][8 k][128, 768] laid side by side
        w_sb = cpool.tile([128, NB * (KPAD // 128) * EMBED], mdt)
        for s in range(NB):
            for k in range(KPAD // 128):
                col = (s * (KPAD // 128) + k) * EMBED
                row = s * KPAD + k * 128
                nc.scalar.dma_start(out=w_sb[:, col:col + EMBED],
                                    in_=w_d[row:row + 128, :])
        bias_sb = cpool.tile([128, EMBED], dt.float32)
        nc.scalar.dma_start(out=bias_sb[:], in_=bias_d[:, :])

        order = [t for t in range(n_chunks) if t // cpb != 0] + \
            [t for t in range(n_chunks) if t // cpb == 0]
        for t in order * reps:
            s = t // cpb
            offs_t = gpool.tile([128, 6], dt.int32, tag="offs")
            nc.sync.dma_start(out=offs_t[:],
                              in_=offs_d[t * 128:(t + 1) * 128, :])
            G5 = gpool.tile([128, KPAD], gdt, tag="G")
            nruns = 4 if s == 0 else 5
            for j in range(nruns):
                nc.gpsimd.indirect_dma_start(
                    out=G5[:, j * RUNL:(j + 1) * RUNL],
                    out_offset=None,
                    in_=q_d[:, :],
                    in_offset=bass.IndirectOffsetOnAxis(
                        ap=offs_t[:, j:j + 1], axis=1),
                )
            nk = 7 if s == 0 else 8          # k=7 all-zero for s=0
            gt = wpool.tile([128, KPAD], mdt, tag="gt")
            for k in range(nk):
                tp = tpool.tile([128, 128], gdt, tag="tp")
                nc.tensor.transpose(
                    out=tp[:], in_=G5[:, k * 128:(k + 1) * 128],
                    identity=ident[:])
                nc.vector.tensor_copy(out=gt[:, k * 128:(k + 1) * 128],
                                      in_=tp[:])
            acc = apool.tile([128, EMBED], dt.float32, tag="acc")
            for k in range(nk):
                lhsT = gt[:, k * 128:(k + 1) * 128]
                wcol = (s * (KPAD // 128) + k) * EMBED
                nc.tensor.matmul(
                    acc[:, 0:512], lhsT,
                    w_sb[:, wcol:wcol + 512],
                    start=(k == 0), stop=(k == nk - 1))
                nc.tensor.matmul(
                    acc[:, 512:EMBED], lhsT,
                    w_sb[:, wcol + 512:wcol + EMBED],
                    start=(k == 0), stop=(k == nk - 1))
            ob = opool.tile([128, EMBED], dt.float32, tag="ob")
            nc.vector.tensor_add(out=ob[:], in0=acc[:], in1=bias_sb[:])
            nc.sync.dma_start(out=out_d[t * 128:(t + 1) * 128, :],
                              in_=ob[:])


def _build(n_cores=NCORES, n_chunks=NCHUNK, cpb=CPB, v_elems=V,
           out_rows=OUTROWS, reps=1):
    import concourse.bass as bass
    import concourse.bacc as bacc
    import concourse.tile as tile
    import concourse.mybir as mybir

    dt = mybir.dt
    nc = bacc.Bacc("TRN2", target_bir_lowering=False, debug=False,
                   num_devices=n_cores)
    aps = {
        "q": nc.dram_tensor("q", [v_elems // 128, 128], dt.float32,
                            kind="ExternalInput").ap(),
        "offs": nc.dram_tensor("offs", [n_chunks * 128, 6], dt.int32,
                               kind="ExternalInput").ap(),
        "w5": nc.dram_tensor("w5", [NB * KPAD, EMBED],
                             dt.bfloat16 if USE_BF16 else dt.float32,
                             kind="ExternalInput").ap(),
        "bias": nc.dram_tensor("bias", [128, EMBED], dt.float32,
                               kind="ExternalInput").ap(),
        "out": nc.dram_tensor("out", [out_rows, EMBED], dt.float32,
                              kind="ExternalOutput").ap(),
    }
    with tile.TileContext(nc) as tc:
        _emit_body(nc, tc, bass, mybir, aps, n_chunks, cpb, reps=reps)
    nc.compile()
    return nc


def _pack_q(x_slice):
    """[BPC, C, H, W] -> quad-interleaved flat Q."""
    q = x_slice.reshape(BPC, C, NQ, 4, W).transpose(0, 2, 4, 1, 3)
    return np.ascontiguousarray(q, dtype=np.float32)  # [BPC, NQ, W, C, 4]


def _w5_variants(proj_w):
    """4 zero-padded weight variants [NB*KPAD, EMBED] f32.

    W5[s][col = j*192 + dw*12 + c*4 + r, e] = proj_w[e, c*256 + ph*16 + dw]
    where ph = 4j + r - s, when 0 <= ph < 16; else 0.
    """
    w5 = np.zeros((NB, KPAD, EMBED), np.float32)
    j = np.arange(5)[:, None, None, None]
    dw = np.arange(16)[None, :, None, None]
    c = np.arange(C)[None, None, :, None]
    r = np.arange(4)[None, None, None, :]
    col = (j * RUNL + dw * 12 + c * 4 + r)          # [5,16,3,4]
    for s in range(NB):
        ph = 4 * j + r - s                          # [5,1,1,4] broadcast
        valid = (ph >= 0) & (ph < PH)
        ph_b, _, _, _ = np.broadcast_arrays(ph, dw, c, r)
        col_b = np.broadcast_to(col, ph_b.shape)
        dw_b = np.broadcast_to(dw, ph_b.shape)
        c_b = np.broadcast_to(c, ph_b.shape)
        v_b = np.broadcast_to(valid, ph_b.shape)
        f_torch = c_b * 256 + ph_b * 16 + dw_b
        sel = v_b.reshape(-1)
        w5[s, col_b.reshape(-1)[sel], :] = proj_w.T[f_torch.reshape(-1)[sel], :]
    return w5.reshape(NB * KPAD, EMBED)


def _slots_for_core(hb, wb):
    """Bucket patches by s=h%4 into 24 chunks of 128 slots.

    Returns offs [SLOTS, 6] int32 (5 gather offsets + out row) and a list
    of (m, h, w, b) overflow patches the caller must compute on host."""
    offs = np.zeros((SLOTS, 6), np.int32)
    # dummies: gather offset 0 (safe), out row = trash region, distinct
    # per partition within a chunk
    offs[:, 5] = M  # dummy marker (filtered on host)
    fill = np.zeros(NB, np.int64)
    overflow = []
    h_flat = hb.reshape(M)
    w_flat = wb.reshape(M)
    for m in range(M):
        h = int(h_flat[m])
        w = int(w_flat[m])
        b = m // N
        s = h % 4
        if fill[s] >= CPB * 128:
            overflow.append((m, h, w, b))
            continue
        slot = s * CPB * 128 + fill[s]
        fill[s] += 1
        q0 = h // 4
        base = (b * NQ + q0) * W + w
        for j in range(5):
            if j == 4 and s == 0:
                offs[slot, j] = 0
            else:
                offs[slot, j] = (base + j * W) * 12
        offs[slot, 5] = m
    return offs, overflow


def _host_patch(x, b_global, h, w, proj_w, proj_b):
    patch = x[b_global, :, h:h + PH, w:w + PW].reshape(-1)
    return patch @ proj_w.T + proj_b


def _in_maps(x, h_idx, w_idx, proj_w, proj_b):
    w5 = _w5_variants(np.asarray(proj_w, np.float32))
    if USE_BF16:
        import ml_dtypes
        w5 = w5.astype(ml_dtypes.bfloat16)
    bias = np.ascontiguousarray(
        np.broadcast_to(np.asarray(proj_b, np.float32), (128, EMBED)))
    maps = []
    overflow_all = []
    for core in range(NCORES):
        xs = np.asarray(x[core * BPC:(core + 1) * BPC], np.float32)
        q = _pack_q(xs).reshape(V // 128, 128)
        hb = np.asarray(h_idx[core * BPC:(core + 1) * BPC])
        wb = np.asarray(w_idx[core * BPC:(core + 1) * BPC])
        offs, overflow = _slots_for_core(hb, wb)
        overflow_all.append(overflow)
        maps.append({"q": q, "offs": offs, "w5": w5, "bias": bias})
    return maps, overflow_all


def _make_runner(nc, n_cores):
    """Jit the prebuilt Bass module once; reuse across calls."""
    import jax
    from jax.sharding import Mesh, PartitionSpec
    from jax.experimental.shard_map import shard_map
    import concourse.mybir as mybir
    from concourse import bass2jax

    bass2jax.install_neuronx_cc_hook()
    in_names, out_names, out_avals, zero_outs = [], [], [], []
    partition_name = (nc.partition_id_tensor.name
                      if nc.partition_id_tensor else None)
    for alloc in nc.m.functions[0].allocations:
        if not isinstance(alloc, mybir.MemoryLocationSet):
            continue
        if not alloc.memorylocations:
            continue
        name = alloc.memorylocations[0].name
        if alloc.kind == "ExternalInput":
            if name != partition_name:
                in_names.append(name)
        elif alloc.kind == "ExternalOutput":
            out_names.append(name)
            shape = tuple(alloc.tensor_shape)
            dtype = mybir.dt.np(alloc.dtype)
            out_avals.append(jax.core.ShapedArray(shape, dtype))
            zero_outs.append(np.zeros(shape, dtype))
    n_params = len(in_names)
    n_outs = len(out_avals)
    all_in_names = list(in_names) + list(out_names)
    if partition_name is not None:
        all_in_names.append(partition_name)
    donate = tuple(range(n_params, n_params + n_outs))

    def _body(*args):
        operands = list(args)
        if partition_name is not None:
            operands.append(bass2jax.partition_id_tensor())
        outs = bass2jax._bass_exec_p.bind(
            *operands,
            out_avals=tuple(out_avals),
            in_names=tuple(all_in_names),
            out_names=tuple(out_names),
            lowering_input_output_aliases=(),
            sim_require_finite=True,
            sim_require_nnan=True,
            nc=nc,
        )
        return tuple(outs)

    devices = jax.devices()[:n_cores]
    mesh = Mesh(np.asarray(devices), ("core",))
    in_specs = (PartitionSpec("core"),) * (n_params + n_outs)
    out_specs = (PartitionSpec("core"),) * n_outs
    jitted = jax.jit(
        shard_map(_body, mesh=mesh, in_specs=in_specs, out_specs=out_specs,
                  check_rep=False),
        donate_argnums=donate, keep_unused=True)

    def run(in_maps):
        per_core = [[np.asarray(m[n]) for n in in_names] for m in in_maps]
        concat_in = [
            np.concatenate([per_core[c][i] for c in range(n_cores)], axis=0)
            for i in range(n_params)]
        concat_zeros = [
            np.zeros((n_cores * z.shape[0], *z.shape[1:]), z.dtype)
            for z in zero_outs]
        outs = jitted(*concat_in, *concat_zeros)
        jax.block_until_ready(outs)
        return [
            {n: np.asarray(outs[i]).reshape(n_cores, *out_avals[i].shape)[c]
             for i, n in enumerate(out_names)}
            for c in range(n_cores)]

    return run


def kernel(**inputs):
    x = np.asarray(inputs["x"])
    h_idx = np.asarray(inputs["h_idx"])
    w_idx = np.asarray(inputs["w_idx"])
    proj_w = np.asarray(inputs["proj_w"])
    proj_b = np.asarray(inputs["proj_b"])

    if "nc" not in _cache:
        _cache["nc"] = _build()
        _cache["run"] = _make_runner(_cache["nc"], NCORES)

    maps, overflow_all = _in_maps(x, h_idx, w_idx, proj_w, proj_b)
    results = _cache["run"](maps)

    out = np.zeros((NCORES, M, EMBED), np.float32)
    for c in range(NCORES):
        rows = maps[c]["offs"][:, 5]
        mask = rows < M
        out[c][rows[mask]] = results[c]["out"][mask]
    out = out.reshape(B, N, EMBED)
    for core, overflow in enumerate(overflow_all):
        for (m, h, w, b) in overflow:
            bg = core * BPC + b
            out[bg, m % N] = _host_patch(x, bg, h, w, proj_w, proj_b)
    return out.astype(np.float32)



# revision 56
# speedup vs baseline: 1.9782x; 1.9782x over previous
"""Trainium2 Bass kernel: CustomPatchEmbedding.

gather 16x16x3 patches at runtime (h_idx, w_idx) + 768x768 linear projection.

kernel(**inputs) takes FULL unsharded inputs
  x [32,3,384,384] f32, h_idx/w_idx [32,576] i32, proj_w [768,768] f32,
  proj_b [768] f32  ->  out [32,576,768] f32.

Sharding: data-parallel batch across 8 NeuronCores (4 images each).

Real-HW SWDGE indirect DMA supports ONE dynamic offset per partition,
streaming the dest free dim contiguously from it (verified by probe; the
CoreSim interpreter is more permissive). The fixed ~1us descriptor-gen cost
per indirect DMA on GpSimd therefore caps throughput at one contiguous run
per partition per instruction.

To make each patch exactly ONE run with zero waste, the host replicates x
16x into w-phase row windows:
    Z[b][w0][h][dw(16)][c(3)] = x[b, c, h, w0 + dw]
so a patch (h, w) is 768 CONTIGUOUS elements at offset
(b*WZ + w)*18432 + 48*h. One indirect DMA per 128-patch chunk gathers the
exact patch data: 18 instructions instead of ~95 (no GpSimd bottleneck),
no s-buckets, no dummy slots, no zero-padded weight variants, K=768.
Patch m = t*128 + p lands in chunk t partition p; K-layout
[row i(16)][dw(16)][c(3)] contracts a single host-permuted weight
Wk[col = i*48+dw*3+c, e] = proj_w[e, c*256+i*16+dw].

Per chunk: one gather (GpSimd), 6 PE transposes (bf16) into one PSUM tile,
one wide ACT copy PSUM->SBUF, 12 bf16 matmuls into split [512]+[256] f32
PSUM banks, two DVE tensor_adds folding bias + bf16 cast, one DMA out.
Chunk t+1's transposes are emitted between chunk t's matmuls so the
transpose->copy->matmul latency hides; gathers prefetch two chunks ahead;
dummy PE transposes in the head finish the 3us p-state clock ramp early.
"""

import numpy as np

PH, PW = 16, 16
EMBED = 768
B, C, H, W = 32, 3, 384, 384
N = 576
NCORES = 8
BPC = B // NCORES            # images per core
M = BPC * N                  # real patches per core (2304)
NCHUNK = M // 128            # 18
K = PH * PW * C              # 768
NKB = K // 128               # 6
WZ = W - PW + 1              # distinct w starts (369)
ROWL = PW * C                # elements per row window (48)
COLL = H * ROWL              # elements per (b, w0) column block (18432)
ZTOT = BPC * WZ * COLL       # 27,205,632 elements
ZPAD = ((ZTOT + 16383) // 16384) * 16384  # flat Z, 16384-aligned

_cache = {}


def _emit_body(nc, tc, bass, mybir):
    dt = mybir.dt
    bf = dt.bfloat16
    z_d = nc.dram_tensor("z", [ZPAD // 16384, 16384], bf,
                         kind="ExternalInput").ap()
    offs_d = nc.dram_tensor("offs", [128, NCHUNK], dt.int32,
                            kind="ExternalInput").ap()
    w_d = nc.dram_tensor("wk", [128, NKB * EMBED], bf,
                         kind="ExternalInput").ap()
    bias_d = nc.dram_tensor("bias", [128, EMBED], bf,
                            kind="ExternalInput").ap()
    out_d = nc.dram_tensor("out", [M, EMBED], bf,
                           kind="ExternalOutput").ap()

    with tc.tile_pool(name="const", bufs=1) as cpool, \
         tc.tile_pool(name="gath", bufs=4) as gpool, \
         tc.tile_pool(name="work", bufs=3) as wpool, \
         tc.tile_pool(name="psumt", bufs=2, space="PSUM") as tpool, \
         tc.tile_pool(name="psuma", bufs=2, space="PSUM") as apool, \
         tc.tile_pool(name="psumw", bufs=1, space="PSUM") as wpsum, \
         tc.tile_pool(name="outp", bufs=3) as opool:
        from concourse.masks import make_identity
        offs_sb = cpool.tile([128, NCHUNK], dt.int32)
        nc.sync.dma_start(out=offs_sb[:], in_=offs_d[:, :])
        # per-k weight tiles in small pipelined loads: matmul k waits only
        # on its own 1.5KB/partition DMA
        w_tiles = [cpool.tile([128, EMBED], bf, name=f"wk{k}")
                   for k in range(NKB)]

        def emit_wload(k):
            nc.scalar.dma_start(out=w_tiles[k][:],
                                in_=w_d[:, k * EMBED:(k + 1) * EMBED])

        emit_wload(0)
        emit_wload(1)
        ident = cpool.tile([128, 128], bf)
        make_identity(nc, ident[:])
        # warmup activation: absorb the 1.3us act-table load in the head
        warm = cpool.tile([128, 1], bf)
        nc.scalar.copy(out=warm[:], in_=ident[:, 0:1])
        for k in range(2, NKB):
            emit_wload(k)
        bias_sb = cpool.tile([128, EMBED], bf)
        nc.sync.dma_start(out=bias_sb[:], in_=bias_d[:, :])
        # PE p-state warmup through the DMA-latency-bound head
        wps = wpsum.tile([128, 128], bf)
        for _ in range(44):
            nc.tensor.transpose(out=wps[:], in_=ident[:], identity=ident[:])

        G_tiles = {}

        def emit_gather(t):
            G = gpool.tile([128, K], bf, tag="G")
            nc.gpsimd.indirect_dma_start(
                out=G[:, :],
                out_offset=None,
                in_=z_d[:, :],
                in_offset=bass.IndirectOffsetOnAxis(
                    ap=offs_sb[:, t:t + 1], axis=1),
            )
            G_tiles[t] = G

        def emit_tc(t):
            """6 PE transposes into one PSUM tile + one wide ACT copy."""
            G = G_tiles.pop(t)
            tp = tpool.tile([128, K], bf, tag="tp")
            for k in range(NKB):
                nc.tensor.transpose(
                    out=tp[:, k * 128:(k + 1) * 128],
                    in_=G[:, k * 128:(k + 1) * 128],
                    identity=ident[:])
            gt = wpool.tile([128, K], bf, tag="gt")
            nc.scalar.copy(out=gt[:], in_=tp[:])
            return gt

        def emit_mm_k(accA, accB, gt, t, k):
            lhsT = gt[:, k * 128:(k + 1) * 128]
            wt = w_tiles[k]
            nc.tensor.matmul(
                accA[:], lhsT, wt[:, 0:512],
                start=(k == 0), stop=(k == NKB - 1))
            nc.tensor.matmul(
                accB[:], lhsT, wt[:, 512:EMBED],
                start=(k == 0), stop=(k == NKB - 1))

        emit_gather(0)
        emit_gather(1)
        cur = emit_tc(0)
        for t in range(NCHUNK):
            if t + 2 < NCHUNK:
                emit_gather(t + 2)
            accA = apool.tile([128, 512], dt.float32, tag="accA")
            accB = apool.tile([128, 256], dt.float32, tag="accB")
            nxt = None
            for k in range(NKB):
                emit_mm_k(accA, accB, cur, t, k)
                if t + 1 < NCHUNK and k == 1:
                    nxt = emit_tc(t + 1)
            if t == NCHUNK - 1:
                oba = opool.tile([128, 512], bf, tag="oba")
                nc.vector.tensor_add(out=oba[:], in0=accA[:],
                                     in1=bias_sb[:, 0:512])
                nc.sync.dma_start(out=out_d[t * 128:(t + 1) * 128, 0:512],
                                  in_=oba[:])
                obb = opool.tile([128, 256], bf, tag="obb")
                nc.vector.tensor_add(out=obb[:], in0=accB[:],
                                     in1=bias_sb[:, 512:])
                nc.sync.dma_start(out=out_d[t * 128:(t + 1) * 128, 512:],
                                  in_=obb[:])
            else:
                ob = opool.tile([128, EMBED], bf, tag="ob")
                nc.vector.tensor_add(out=ob[:, 0:512], in0=accA[:],
                                     in1=bias_sb[:, 0:512])
                nc.vector.tensor_add(out=ob[:, 512:], in0=accB[:],
                                     in1=bias_sb[:, 512:])
                nc.sync.dma_start(out=out_d[t * 128:(t + 1) * 128, :],
                                  in_=ob[:])
            cur = nxt


def _build(n_cores=NCORES):
    import concourse.bass as bass
    import concourse.bacc as bacc
    import concourse.tile as tile
    import concourse.mybir as mybir

    nc = bacc.Bacc("TRN2", target_bir_lowering=False, debug=False,
                   num_devices=n_cores)
    with tile.TileContext(nc) as tc:
        _emit_body(nc, tc, bass, mybir)
    nc.compile()
    return nc


def _pack_z(x_slice):
    """[BPC, C, H, W] f32 -> replicated w-phase row-window Z, flat bf16.

    Z[b][w0][h][dw][c] = x[b, c, h, w0+dw]; 16x replication makes a patch
    (h, w) one contiguous 768-element run at (b*WZ+w)*COLL + 48*h."""
    import ml_dtypes
    from numpy.lib.stride_tricks import sliding_window_view
    r = x_slice.transpose(0, 2, 3, 1)            # [b, h, w, c]
    sw = sliding_window_view(r, PW, axis=2)      # [b, h, w0, c, dw]
    z = sw.transpose(0, 2, 1, 4, 3)              # [b, w0, h, dw, c]
    flat = np.zeros(ZPAD, dtype=ml_dtypes.bfloat16)
    flat[:ZTOT] = np.ascontiguousarray(z, dtype=np.float32).reshape(-1) \
        .astype(ml_dtypes.bfloat16)
    return flat.reshape(ZPAD // 16384, 16384)


def _wk_perm(proj_w):
    """single permuted weight [128, 6*768] bf16:
    Wk[col = i*48 + dw*3 + c, e] = proj_w[e, c*256 + i*16 + dw]."""
    import ml_dtypes
    col = np.arange(K)
    i, rem = col // ROWL, col % ROWL
    dw, c = rem // C, rem % C
    f = c * 256 + i * PW + dw
    wk = proj_w.T[f]                             # [768 col, 768 e]
    wk = wk.reshape(NKB, 128, EMBED).transpose(1, 0, 2).reshape(128, -1)
    return np.ascontiguousarray(wk.astype(ml_dtypes.bfloat16))


def _host_patch(x, b_global, h, w, proj_w, proj_b):
    patch = x[b_global, :, h:h + PH, w:w + PW].reshape(-1)
    return patch @ proj_w.T + proj_b


def _in_maps(x, h_idx, w_idx, proj_w, proj_b):
    import ml_dtypes
    wk = _wk_perm(np.asarray(proj_w, np.float32))
    bias = np.ascontiguousarray(
        np.broadcast_to(np.asarray(proj_b, np.float32), (128, EMBED))
        .astype(ml_dtypes.bfloat16))

    # Z packing is 16x-replicated (~54MB/core) and depends only on x: cache
    if _cache.get("z_x") is not None and np.array_equal(_cache["z_x"], x):
        z_list = _cache["z_list"]
    else:
        z_list = [
            _pack_z(np.asarray(x[core * BPC:(core + 1) * BPC], np.float32))
            for core in range(NCORES)]
        _cache["z_x"] = np.array(x, copy=True)
        _cache["z_list"] = z_list

    maps = []
    for core in range(NCORES):
        hb = np.asarray(h_idx[core * BPC:(core + 1) * BPC]).reshape(M)
        wb = np.asarray(w_idx[core * BPC:(core + 1) * BPC]).reshape(M)
        b = np.arange(M) // N
        offs = (b * WZ + wb.astype(np.int64)) * COLL + ROWL * hb
        offs = np.ascontiguousarray(
            offs.reshape(NCHUNK, 128).T.astype(np.int32))
        maps.append({"z": z_list[core], "offs": offs, "wk": wk,
                     "bias": bias})
    return maps


def _make_runner(nc, n_cores):
    """Jit the prebuilt Bass module once; reuse across calls."""
    import jax
    from jax.sharding import Mesh, PartitionSpec
    from jax.experimental.shard_map import shard_map
    import concourse.mybir as mybir
    from concourse import bass2jax

    bass2jax.install_neuronx_cc_hook()
    in_names, out_names, out_avals, zero_outs = [], [], [], []
    partition_name = (nc.partition_id_tensor.name
                      if nc.partition_id_tensor else None)
    for alloc in nc.m.functions[0].allocations:
        if not isinstance(alloc, mybir.MemoryLocationSet):
            continue
        if not alloc.memorylocations:
            continue
        name = alloc.memorylocations[0].name
        if alloc.kind == "ExternalInput":
            if name != partition_name:
                in_names.append(name)
        elif alloc.kind == "ExternalOutput":
            out_names.append(name)
            shape = tuple(alloc.tensor_shape)
            dtype = mybir.dt.np(alloc.dtype)
            out_avals.append(jax.core.ShapedArray(shape, dtype))
            zero_outs.append(np.zeros(shape, dtype))
    n_params = len(in_names)
    n_outs = len(out_avals)
    all_in_names = list(in_names) + list(out_names)
    if partition_name is not None:
        all_in_names.append(partition_name)
    donate = tuple(range(n_params, n_params + n_outs))

    def _body(*args):
        operands = list(args)
        if partition_name is not None:
            operands.append(bass2jax.partition_id_tensor())
        outs = bass2jax._bass_exec_p.bind(
            *operands,
            out_avals=tuple(out_avals),
            in_names=tuple(all_in_names),
            out_names=tuple(out_names),
            lowering_input_output_aliases=(),
            sim_require_finite=True,
            sim_require_nnan=True,
            nc=nc,
        )
        return tuple(outs)

    devices = jax.devices()[:n_cores]
    mesh = Mesh(np.asarray(devices), ("core",))
    in_specs = (PartitionSpec("core"),) * (n_params + n_outs)
    out_specs = (PartitionSpec("core"),) * n_outs
    jitted = jax.jit(
        shard_map(_body, mesh=mesh, in_specs=in_specs, out_specs=out_specs,
                  check_rep=False),
        donate_argnums=donate, keep_unused=True)

    def run(in_maps):
        per_core = [[np.asarray(m[n]) for n in in_names] for m in in_maps]
        concat_in = [
            np.concatenate([per_core[c][i] for c in range(n_cores)], axis=0)
            for i in range(n_params)]
        concat_zeros = [
            np.zeros((n_cores * z.shape[0], *z.shape[1:]), z.dtype)
            for z in zero_outs]
        outs = jitted(*concat_in, *concat_zeros)
        jax.block_until_ready(outs)
        return [
            {n: np.asarray(outs[i]).reshape(n_cores, *out_avals[i].shape)[c]
             for i, n in enumerate(out_names)}
            for c in range(n_cores)]

    return run


def kernel(**inputs):
    x = np.asarray(inputs["x"])
    h_idx = np.asarray(inputs["h_idx"])
    w_idx = np.asarray(inputs["w_idx"])
    proj_w = np.asarray(inputs["proj_w"], np.float32)
    proj_b = np.asarray(inputs["proj_b"], np.float32)

    if "nc" not in _cache:
        _cache["nc"] = _build()
        _cache["run"] = _make_runner(_cache["nc"], NCORES)

    maps = _in_maps(x, h_idx, w_idx, proj_w, proj_b)
    results = _cache["run"](maps)

    out = np.stack([results[c]["out"].astype(np.float32)
                    for c in range(NCORES)])
    return np.ascontiguousarray(out.reshape(B, N, EMBED))


# revision 57
# speedup vs baseline: 2.0001x; 1.0111x over previous
"""Trainium2 Bass kernel: CustomPatchEmbedding.

gather 16x16x3 patches at runtime (h_idx, w_idx) + 768x768 linear projection.

kernel(**inputs) takes FULL unsharded inputs
  x [32,3,384,384] f32, h_idx/w_idx [32,576] i32, proj_w [768,768] f32,
  proj_b [768] f32  ->  out [32,576,768] f32.

Sharding: data-parallel batch across 8 NeuronCores (4 images each).

Real-HW SWDGE indirect DMA supports ONE dynamic offset per partition,
streaming the dest free dim contiguously from it (verified by probe; the
CoreSim interpreter is more permissive). The fixed ~1us descriptor-gen cost
per indirect DMA on GpSimd therefore caps throughput at one contiguous run
per partition per instruction.

To make each patch exactly ONE run with zero waste, the host replicates x
16x into w-phase row windows:
    Z[b][w0][h][dw(16)][c(3)] = x[b, c, h, w0 + dw]
so a patch (h, w) is 768 CONTIGUOUS elements at offset
(b*WZ + w)*18432 + 48*h. One indirect DMA per 128-patch chunk gathers the
exact patch data: 18 instructions instead of ~95 (no GpSimd bottleneck),
no s-buckets, no dummy slots, no zero-padded weight variants, K=768.
Patch m = t*128 + p lands in chunk t partition p; K-layout
[row i(16)][dw(16)][c(3)] contracts a single host-permuted weight
Wk[col = i*48+dw*3+c, e] = proj_w[e, c*256+i*16+dw].

Per chunk: one gather (GpSimd), 6 PE transposes (bf16) into one PSUM tile,
one wide ACT copy PSUM->SBUF, 12 bf16 matmuls into split [512]+[256] f32
PSUM banks, two DVE tensor_adds folding bias + bf16 cast, one DMA out.
Chunk t+1's transposes are emitted between chunk t's matmuls so the
transpose->copy->matmul latency hides; gathers prefetch two chunks ahead;
dummy PE transposes in the head finish the 3us p-state clock ramp early.
"""

import numpy as np

PH, PW = 16, 16
EMBED = 768
B, C, H, W = 32, 3, 384, 384
N = 576
NCORES = 8
BPC = B // NCORES            # images per core
M = BPC * N                  # real patches per core (2304)
NCHUNK = M // 128            # 18
K = PH * PW * C              # 768
NKB = K // 128               # 6
WZ = W - PW + 1              # distinct w starts (369)
ROWL = PW * C                # elements per row window (48)
COLL = H * ROWL              # elements per (b, w0) column block (18432)
ZTOT = BPC * WZ * COLL       # 27,205,632 elements
ZPAD = ((ZTOT + 16383) // 16384) * 16384  # flat Z, 16384-aligned

_cache = {}


def _emit_body(nc, tc, bass, mybir):
    dt = mybir.dt
    bf = dt.bfloat16
    z_d = nc.dram_tensor("z", [ZPAD // 16384, 16384], bf,
                         kind="ExternalInput").ap()
    offs_d = nc.dram_tensor("offs", [128, NCHUNK], dt.int32,
                            kind="ExternalInput").ap()
    w_d = nc.dram_tensor("wk", [128, NKB * EMBED], bf,
                         kind="ExternalInput").ap()
    bias_d = nc.dram_tensor("bias", [128, EMBED], bf,
                            kind="ExternalInput").ap()
    out_d = nc.dram_tensor("out", [M, EMBED], bf,
                           kind="ExternalOutput").ap()

    with tc.tile_pool(name="const", bufs=1) as cpool, \
         tc.tile_pool(name="gath", bufs=4) as gpool, \
         tc.tile_pool(name="work", bufs=3) as wpool, \
         tc.tile_pool(name="psumt", bufs=2, space="PSUM") as tpool, \
         tc.tile_pool(name="psuma", bufs=2, space="PSUM") as apool, \
         tc.tile_pool(name="psumw", bufs=1, space="PSUM") as wpsum, \
         tc.tile_pool(name="outp", bufs=3) as opool:
        from concourse.masks import make_identity
        offs_sb = cpool.tile([128, NCHUNK], dt.int32)
        nc.sync.dma_start(out=offs_sb[:], in_=offs_d[:, :])
        # per-k weight tiles in small pipelined loads: matmul k waits only
        # on its own 1.5KB/partition DMA
        w_tiles = [cpool.tile([128, EMBED], bf, name=f"wk{k}")
                   for k in range(NKB)]

        def emit_wload(k):
            nc.scalar.dma_start(out=w_tiles[k][:],
                                in_=w_d[:, k * EMBED:(k + 1) * EMBED])

        emit_wload(0)
        emit_wload(1)
        ident = cpool.tile([128, 128], bf)
        make_identity(nc, ident[:])
        # warmup activation: absorb the 1.3us act-table load in the head
        warm = cpool.tile([128, 1], bf)
        nc.scalar.copy(out=warm[:], in_=ident[:, 0:1])
        for k in range(2, NKB):
            emit_wload(k)
        bias_sb = cpool.tile([128, EMBED], bf)
        nc.sync.dma_start(out=bias_sb[:], in_=bias_d[:, :])
        # PE p-state warmup through the DMA-latency-bound head
        wps = wpsum.tile([128, 128], bf)
        for _ in range(44):
            nc.tensor.transpose(out=wps[:], in_=ident[:], identity=ident[:])

        G_tiles = {}

        def emit_gather(t):
            G = gpool.tile([128, K], bf, tag="G")
            nc.gpsimd.indirect_dma_start(
                out=G[:, :],
                out_offset=None,
                in_=z_d[:, :],
                in_offset=bass.IndirectOffsetOnAxis(
                    ap=offs_sb[:, t:t + 1], axis=1),
            )
            G_tiles[t] = G

        def emit_tc(t):
            """6 PE transposes into one PSUM tile + one wide ACT copy."""
            G = G_tiles.pop(t)
            tp = tpool.tile([128, K], bf, tag="tp")
            for k in range(NKB):
                nc.tensor.transpose(
                    out=tp[:, k * 128:(k + 1) * 128],
                    in_=G[:, k * 128:(k + 1) * 128],
                    identity=ident[:])
            gt = wpool.tile([128, K], bf, tag="gt")
            nc.scalar.copy(out=gt[:], in_=tp[:])
            return gt

        def emit_mm(acc, gt, k, lo, hi):
            nc.tensor.matmul(
                acc[:], gt[:, k * 128:(k + 1) * 128],
                w_tiles[k][:, lo:hi],
                start=(k == 0), stop=(k == NKB - 1))

        emit_gather(0)
        emit_gather(1)
        cur = emit_tc(0)
        for t in range(NCHUNK):
            if t + 2 < NCHUNK:
                emit_gather(t + 2)
            accA = apool.tile([128, 512], dt.float32, tag="accA")
            accB = apool.tile([128, 256], dt.float32, tag="accB")
            # all A-column matmuls first, then all B: accA's accumulation
            # group closes ~1.3us before accB's, so the A-half bias-add and
            # store overlap the B-half matmuls (shrinks the drain tail)
            nxt = None
            for k in range(NKB):
                emit_mm(accA, cur, k, 0, 512)
                if t + 1 < NCHUNK and k == 1:
                    nxt = emit_tc(t + 1)
            for k in range(NKB):
                emit_mm(accB, cur, k, 512, EMBED)
            if t == NCHUNK - 1:
                oba = opool.tile([128, 512], bf, tag="oba")
                nc.vector.tensor_add(out=oba[:], in0=accA[:],
                                     in1=bias_sb[:, 0:512])
                nc.sync.dma_start(out=out_d[t * 128:(t + 1) * 128, 0:512],
                                  in_=oba[:])
                obb = opool.tile([128, 256], bf, tag="obb")
                nc.vector.tensor_add(out=obb[:], in0=accB[:],
                                     in1=bias_sb[:, 512:])
                nc.scalar.dma_start(out=out_d[t * 128:(t + 1) * 128, 512:],
                                    in_=obb[:])
            else:
                ob = opool.tile([128, EMBED], bf, tag="ob")
                nc.vector.tensor_add(out=ob[:, 0:512], in0=accA[:],
                                     in1=bias_sb[:, 0:512])
                nc.vector.tensor_add(out=ob[:, 512:], in0=accB[:],
                                     in1=bias_sb[:, 512:])
                nc.sync.dma_start(out=out_d[t * 128:(t + 1) * 128, :],
                                  in_=ob[:])
            cur = nxt


def _build(n_cores=NCORES):
    import concourse.bass as bass
    import concourse.bacc as bacc
    import concourse.tile as tile
    import concourse.mybir as mybir

    nc = bacc.Bacc("TRN2", target_bir_lowering=False, debug=False,
                   num_devices=n_cores)
    with tile.TileContext(nc) as tc:
        _emit_body(nc, tc, bass, mybir)
    nc.compile()
    return nc


def _pack_z(x_slice):
    """[BPC, C, H, W] f32 -> replicated w-phase row-window Z, flat bf16.

    Z[b][w0][h][dw][c] = x[b, c, h, w0+dw]; 16x replication makes a patch
    (h, w) one contiguous 768-element run at (b*WZ+w)*COLL + 48*h."""
    import ml_dtypes
    from numpy.lib.stride_tricks import sliding_window_view
    r = x_slice.transpose(0, 2, 3, 1)            # [b, h, w, c]
    sw = sliding_window_view(r, PW, axis=2)      # [b, h, w0, c, dw]
    z = sw.transpose(0, 2, 1, 4, 3)              # [b, w0, h, dw, c]
    flat = np.zeros(ZPAD, dtype=ml_dtypes.bfloat16)
    flat[:ZTOT] = np.ascontiguousarray(z, dtype=np.float32).reshape(-1) \
        .astype(ml_dtypes.bfloat16)
    return flat.reshape(ZPAD // 16384, 16384)


def _wk_perm(proj_w):
    """single permuted weight [128, 6*768] bf16:
    Wk[col = i*48 + dw*3 + c, e] = proj_w[e, c*256 + i*16 + dw]."""
    import ml_dtypes
    col = np.arange(K)
    i, rem = col // ROWL, col % ROWL
    dw, c = rem // C, rem % C
    f = c * 256 + i * PW + dw
    wk = proj_w.T[f]                             # [768 col, 768 e]
    wk = wk.reshape(NKB, 128, EMBED).transpose(1, 0, 2).reshape(128, -1)
    return np.ascontiguousarray(wk.astype(ml_dtypes.bfloat16))


def _host_patch(x, b_global, h, w, proj_w, proj_b):
    patch = x[b_global, :, h:h + PH, w:w + PW].reshape(-1)
    return patch @ proj_w.T + proj_b


def _in_maps(x, h_idx, w_idx, proj_w, proj_b):
    import ml_dtypes
    wk = _wk_perm(np.asarray(proj_w, np.float32))
    bias = np.ascontiguousarray(
        np.broadcast_to(np.asarray(proj_b, np.float32), (128, EMBED))
        .astype(ml_dtypes.bfloat16))

    # Z packing is 16x-replicated (~54MB/core) and depends only on x: cache
    if _cache.get("z_x") is not None and np.array_equal(_cache["z_x"], x):
        z_list = _cache["z_list"]
    else:
        z_list = [
            _pack_z(np.asarray(x[core * BPC:(core + 1) * BPC], np.float32))
            for core in range(NCORES)]
        _cache["z_x"] = np.array(x, copy=True)
        _cache["z_list"] = z_list

    maps = []
    for core in range(NCORES):
        hb = np.asarray(h_idx[core * BPC:(core + 1) * BPC]).reshape(M)
        wb = np.asarray(w_idx[core * BPC:(core + 1) * BPC]).reshape(M)
        b = np.arange(M) // N
        offs = (b * WZ + wb.astype(np.int64)) * COLL + ROWL * hb
        offs = np.ascontiguousarray(
            offs.reshape(NCHUNK, 128).T.astype(np.int32))
        maps.append({"z": z_list[core], "offs": offs, "wk": wk,
                     "bias": bias})
    return maps


def _make_runner(nc, n_cores):
    """Jit the prebuilt Bass module once; reuse across calls."""
    import jax
    from jax.sharding import Mesh, PartitionSpec
    from jax.experimental.shard_map import shard_map
    import concourse.mybir as mybir
    from concourse import bass2jax

    bass2jax.install_neuronx_cc_hook()
    in_names, out_names, out_avals, zero_outs = [], [], [], []
    partition_name = (nc.partition_id_tensor.name
                      if nc.partition_id_tensor else None)
    for alloc in nc.m.functions[0].allocations:
        if not isinstance(alloc, mybir.MemoryLocationSet):
            continue
        if not alloc.memorylocations:
            continue
        name = alloc.memorylocations[0].name
        if alloc.kind == "ExternalInput":
            if name != partition_name:
                in_names.append(name)
        elif alloc.kind == "ExternalOutput":
            out_names.append(name)
            shape = tuple(alloc.tensor_shape)
            dtype = mybir.dt.np(alloc.dtype)
            out_avals.append(jax.core.ShapedArray(shape, dtype))
            zero_outs.append(np.zeros(shape, dtype))
    n_params = len(in_names)
    n_outs = len(out_avals)
    all_in_names = list(in_names) + list(out_names)
    if partition_name is not None:
        all_in_names.append(partition_name)
    donate = tuple(range(n_params, n_params + n_outs))

    def _body(*args):
        operands = list(args)
        if partition_name is not None:
            operands.append(bass2jax.partition_id_tensor())
        outs = bass2jax._bass_exec_p.bind(
            *operands,
            out_avals=tuple(out_avals),
            in_names=tuple(all_in_names),
            out_names=tuple(out_names),
            lowering_input_output_aliases=(),
            sim_require_finite=True,
            sim_require_nnan=True,
            nc=nc,
        )
        return tuple(outs)

    devices = jax.devices()[:n_cores]
    mesh = Mesh(np.asarray(devices), ("core",))
    in_specs = (PartitionSpec("core"),) * (n_params + n_outs)
    out_specs = (PartitionSpec("core"),) * n_outs
    jitted = jax.jit(
        shard_map(_body, mesh=mesh, in_specs=in_specs, out_specs=out_specs,
                  check_rep=False),
        donate_argnums=donate, keep_unused=True)

    def run(in_maps):
        per_core = [[np.asarray(m[n]) for n in in_names] for m in in_maps]
        concat_in = [
            np.concatenate([per_core[c][i] for c in range(n_cores)], axis=0)
            for i in range(n_params)]
        concat_zeros = [
            np.zeros((n_cores * z.shape[0], *z.shape[1:]), z.dtype)
            for z in zero_outs]
        outs = jitted(*concat_in, *concat_zeros)
        jax.block_until_ready(outs)
        return [
            {n: np.asarray(outs[i]).reshape(n_cores, *out_avals[i].shape)[c]
             for i, n in enumerate(out_names)}
            for c in range(n_cores)]

    return run


def kernel(**inputs):
    x = np.asarray(inputs["x"])
    h_idx = np.asarray(inputs["h_idx"])
    w_idx = np.asarray(inputs["w_idx"])
    proj_w = np.asarray(inputs["proj_w"], np.float32)
    proj_b = np.asarray(inputs["proj_b"], np.float32)

    if "nc" not in _cache:
        _cache["nc"] = _build()
        _cache["run"] = _make_runner(_cache["nc"], NCORES)

    maps = _in_maps(x, h_idx, w_idx, proj_w, proj_b)
    results = _cache["run"](maps)

    out = np.stack([results[c]["out"].astype(np.float32)
                    for c in range(NCORES)])
    return np.ascontiguousarray(out.reshape(B, N, EMBED))


# revision 61
# speedup vs baseline: 2.0132x; 1.0066x over previous
"""Trainium2 Bass kernel: CustomPatchEmbedding.

gather 16x16x3 patches at runtime (h_idx, w_idx) + 768x768 linear projection.

kernel(**inputs) takes FULL unsharded inputs
  x [32,3,384,384] f32, h_idx/w_idx [32,576] i32, proj_w [768,768] f32,
  proj_b [768] f32  ->  out [32,576,768] f32.

Sharding: data-parallel batch across 8 NeuronCores (4 images each).

Real-HW SWDGE indirect DMA supports ONE dynamic offset per partition,
streaming the dest free dim contiguously from it (verified by probe; the
CoreSim interpreter is more permissive). The fixed ~1us descriptor-gen cost
per indirect DMA on GpSimd therefore caps throughput at one contiguous run
per partition per instruction.

To make each patch exactly ONE run with zero waste, the host replicates x
16x into w-phase row windows:
    Z[b][w0][h][dw(16)][c(3)] = x[b, c, h, w0 + dw]
so a patch (h, w) is 768 CONTIGUOUS elements at offset
(b*WZ + w)*18432 + 48*h. One indirect DMA per 128-patch chunk gathers the
exact patch data: 18 instructions instead of ~95 (no GpSimd bottleneck),
no s-buckets, no dummy slots, no zero-padded weight variants, K=768.
Patch m = t*128 + p lands in chunk t partition p; K-layout
[row i(16)][dw(16)][c(3)] contracts a single host-permuted weight
Wk[col = i*48+dw*3+c, e] = proj_w[e, c*256+i*16+dw].

Per chunk: one gather (GpSimd), 6 PE transposes (bf16) into one PSUM tile,
one wide ACT copy PSUM->SBUF, 12 bf16 matmuls into split [512]+[256] f32
PSUM banks, two DVE tensor_adds folding bias + bf16 cast, one DMA out.
Chunk t+1's transposes are emitted between chunk t's matmuls so the
transpose->copy->matmul latency hides; gathers prefetch two chunks ahead;
dummy PE transposes in the head finish the 3us p-state clock ramp early.
"""

import numpy as np

PH, PW = 16, 16
EMBED = 768
B, C, H, W = 32, 3, 384, 384
N = 576
NCORES = 8
BPC = B // NCORES            # images per core
M = BPC * N                  # real patches per core (2304)
NCHUNK = M // 128            # 18
K = PH * PW * C              # 768
NKB = K // 128               # 6
WZ = W - PW + 1              # distinct w starts (369)
ROWL = PW * C                # elements per row window (48)
COLL = H * ROWL              # elements per (b, w0) column block (18432)
ZTOT = BPC * WZ * COLL       # 27,205,632 elements
ZPAD = ((ZTOT + 16383) // 16384) * 16384  # flat Z, 16384-aligned

_cache = {}


def _emit_body(nc, tc, bass, mybir):
    dt = mybir.dt
    bf = dt.bfloat16
    z_d = nc.dram_tensor("z", [ZPAD // 16384, 16384], bf,
                         kind="ExternalInput").ap()
    offs_d = nc.dram_tensor("offs", [128, NCHUNK], dt.int32,
                            kind="ExternalInput").ap()
    w_d = nc.dram_tensor("wk", [128, NKB * EMBED], bf,
                         kind="ExternalInput").ap()
    bias_d = nc.dram_tensor("bias", [128, EMBED], bf,
                            kind="ExternalInput").ap()
    out_d = nc.dram_tensor("out", [M, EMBED], bf,
                           kind="ExternalOutput").ap()

    with tc.tile_pool(name="const", bufs=1) as cpool, \
         tc.tile_pool(name="gath", bufs=4) as gpool, \
         tc.tile_pool(name="work", bufs=3) as wpool, \
         tc.tile_pool(name="psumt", bufs=2, space="PSUM") as tpool, \
         tc.tile_pool(name="psuma", bufs=2, space="PSUM") as apool, \
         tc.tile_pool(name="psumw", bufs=1, space="PSUM") as wpsum, \
         tc.tile_pool(name="outp", bufs=3) as opool:
        from concourse.masks import make_identity
        offs_sb = cpool.tile([128, NCHUNK], dt.int32)
        nc.sync.dma_start(out=offs_sb[:], in_=offs_d[:, :])
        # per-k weight tiles in small pipelined loads: matmul k waits only
        # on its own 1.5KB/partition DMA
        w_tiles = [cpool.tile([128, EMBED], bf, name=f"wk{k}")
                   for k in range(NKB)]

        def emit_wload(k):
            nc.scalar.dma_start(out=w_tiles[k][:],
                                in_=w_d[:, k * EMBED:(k + 1) * EMBED])

        emit_wload(0)
        emit_wload(1)
        ident = cpool.tile([128, 128], bf)
        make_identity(nc, ident[:])
        # warmup activation: absorb the 1.3us act-table load in the head
        warm = cpool.tile([128, 1], bf)
        nc.scalar.copy(out=warm[:], in_=ident[:, 0:1])
        for k in range(2, NKB):
            emit_wload(k)
        bias_sb = cpool.tile([128, EMBED], bf)
        nc.sync.dma_start(out=bias_sb[:], in_=bias_d[:, :])
        # PE p-state warmup through the DMA-latency-bound head
        wps = wpsum.tile([128, 128], bf)
        for _ in range(44):
            nc.tensor.transpose(out=wps[:], in_=ident[:], identity=ident[:])

        G_tiles = {}

        def emit_gather(t):
            G = gpool.tile([128, K], bf, tag="G")
            nc.gpsimd.indirect_dma_start(
                out=G[:, :],
                out_offset=None,
                in_=z_d[:, :],
                in_offset=bass.IndirectOffsetOnAxis(
                    ap=offs_sb[:, t:t + 1], axis=1),
            )
            G_tiles[t] = G

        def emit_tc(t):
            """6 PE transposes into one PSUM tile + copy PSUM->SBUF.

            Chunk 0's copy is on the critical head path: split it into
            parallel ACT/DVE halves in separate tiles so matmul k=0 waits
            only ~390ns for the first half instead of 712ns for all of it."""
            G = G_tiles.pop(t)
            tp = tpool.tile([128, K], bf, tag="tp")
            for k in range(NKB):
                nc.tensor.transpose(
                    out=tp[:, k * 128:(k + 1) * 128],
                    in_=G[:, k * 128:(k + 1) * 128],
                    identity=ident[:])
            if t == 0:
                gta = wpool.tile([128, K // 2], bf, tag="gt0a")
                nc.scalar.copy(out=gta[:], in_=tp[:, 0:K // 2])
                gtb = wpool.tile([128, K // 2], bf, tag="gt0b")
                nc.vector.tensor_copy(out=gtb[:], in_=tp[:, K // 2:])
                return (gta, gtb)
            gt = wpool.tile([128, K], bf, tag="gt")
            nc.scalar.copy(out=gt[:], in_=tp[:])
            return gt

        def emit_mm(acc, gt, k, lo, hi):
            if isinstance(gt, tuple):
                half = gt[k // (NKB // 2)]
                lhsT = half[:, (k % (NKB // 2)) * 128:
                            (k % (NKB // 2) + 1) * 128]
            else:
                lhsT = gt[:, k * 128:(k + 1) * 128]
            nc.tensor.matmul(
                acc[:], lhsT, w_tiles[k][:, lo:hi],
                start=(k == 0), stop=(k == NKB - 1))

        emit_gather(0)
        emit_gather(1)
        cur = emit_tc(0)
        for t in range(NCHUNK):
            if t + 2 < NCHUNK:
                emit_gather(t + 2)
            accA = apool.tile([128, 512], dt.float32, tag="accA")
            accB = apool.tile([128, 256], dt.float32, tag="accB")
            # all A-column matmuls first, then all B: accA's accumulation
            # group closes ~1.3us before accB's, so the A-half bias-add and
            # store overlap the B-half matmuls (shrinks the drain tail)
            nxt = None
            for k in range(NKB):
                emit_mm(accA, cur, k, 0, 512)
                if t + 1 < NCHUNK and k == 1:
                    nxt = emit_tc(t + 1)
            for k in range(NKB):
                emit_mm(accB, cur, k, 512, EMBED)
            if t == NCHUNK - 1:
                oba = opool.tile([128, 512], bf, tag="oba")
                nc.vector.tensor_add(out=oba[:], in0=accA[:],
                                     in1=bias_sb[:, 0:512])
                nc.sync.dma_start(out=out_d[t * 128:(t + 1) * 128, 0:512],
                                  in_=oba[:])
                obb = opool.tile([128, 256], bf, tag="obb")
                nc.vector.tensor_add(out=obb[:], in0=accB[:],
                                     in1=bias_sb[:, 512:])
                nc.scalar.dma_start(out=out_d[t * 128:(t + 1) * 128, 512:],
                                    in_=obb[:])
            else:
                ob = opool.tile([128, EMBED], bf, tag="ob")
                nc.vector.tensor_add(out=ob[:, 0:512], in0=accA[:],
                                     in1=bias_sb[:, 0:512])
                nc.vector.tensor_add(out=ob[:, 512:], in0=accB[:],
                                     in1=bias_sb[:, 512:])
                nc.sync.dma_start(out=out_d[t * 128:(t + 1) * 128, :],
                                  in_=ob[:])
            cur = nxt


def _build(n_cores=NCORES):
    import concourse.bass as bass
    import concourse.bacc as bacc
    import concourse.tile as tile
    import concourse.mybir as mybir

    nc = bacc.Bacc("TRN2", target_bir_lowering=False, debug=False,
                   num_devices=n_cores)
    with tile.TileContext(nc) as tc:
        _emit_body(nc, tc, bass, mybir)
    nc.compile()
    return nc


def _pack_z(x_slice):
    """[BPC, C, H, W] f32 -> replicated w-phase row-window Z, flat bf16.

    Z[b][w0][h][dw][c] = x[b, c, h, w0+dw]; 16x replication makes a patch
    (h, w) one contiguous 768-element run at (b*WZ+w)*COLL + 48*h."""
    import ml_dtypes
    from numpy.lib.stride_tricks import sliding_window_view
    r = x_slice.transpose(0, 2, 3, 1)            # [b, h, w, c]
    sw = sliding_window_view(r, PW, axis=2)      # [b, h, w0, c, dw]
    z = sw.transpose(0, 2, 1, 4, 3)              # [b, w0, h, dw, c]
    flat = np.zeros(ZPAD, dtype=ml_dtypes.bfloat16)
    flat[:ZTOT] = np.ascontiguousarray(z, dtype=np.float32).reshape(-1) \
        .astype(ml_dtypes.bfloat16)
    return flat.reshape(ZPAD // 16384, 16384)


def _wk_perm(proj_w):
    """single permuted weight [128, 6*768] bf16:
    Wk[col = i*48 + dw*3 + c, e] = proj_w[e, c*256 + i*16 + dw]."""
    import ml_dtypes
    col = np.arange(K)
    i, rem = col // ROWL, col % ROWL
    dw, c = rem // C, rem % C
    f = c * 256 + i * PW + dw
    wk = proj_w.T[f]                             # [768 col, 768 e]
    wk = wk.reshape(NKB, 128, EMBED).transpose(1, 0, 2).reshape(128, -1)
    return np.ascontiguousarray(wk.astype(ml_dtypes.bfloat16))


def _host_patch(x, b_global, h, w, proj_w, proj_b):
    patch = x[b_global, :, h:h + PH, w:w + PW].reshape(-1)
    return patch @ proj_w.T + proj_b


def _in_maps(x, h_idx, w_idx, proj_w, proj_b):
    import ml_dtypes
    wk = _wk_perm(np.asarray(proj_w, np.float32))
    bias = np.ascontiguousarray(
        np.broadcast_to(np.asarray(proj_b, np.float32), (128, EMBED))
        .astype(ml_dtypes.bfloat16))

    # Z packing is 16x-replicated (~54MB/core) and depends only on x: cache
    if _cache.get("z_x") is not None and np.array_equal(_cache["z_x"], x):
        z_list = _cache["z_list"]
    else:
        z_list = [
            _pack_z(np.asarray(x[core * BPC:(core + 1) * BPC], np.float32))
            for core in range(NCORES)]
        _cache["z_x"] = np.array(x, copy=True)
        _cache["z_list"] = z_list

    maps = []
    for core in range(NCORES):
        hb = np.asarray(h_idx[core * BPC:(core + 1) * BPC]).reshape(M)
        wb = np.asarray(w_idx[core * BPC:(core + 1) * BPC]).reshape(M)
        b = np.arange(M) // N
        offs = (b * WZ + wb.astype(np.int64)) * COLL + ROWL * hb
        offs = np.ascontiguousarray(
            offs.reshape(NCHUNK, 128).T.astype(np.int32))
        maps.append({"z": z_list[core], "offs": offs, "wk": wk,
                     "bias": bias})
    return maps


def _make_runner(nc, n_cores):
    """Jit the prebuilt Bass module once; reuse across calls."""
    import jax
    from jax.sharding import Mesh, PartitionSpec
    from jax.experimental.shard_map import shard_map
    import concourse.mybir as mybir
    from concourse import bass2jax

    bass2jax.install_neuronx_cc_hook()
    in_names, out_names, out_avals, zero_outs = [], [], [], []
    partition_name = (nc.partition_id_tensor.name
                      if nc.partition_id_tensor else None)
    for alloc in nc.m.functions[0].allocations:
        if not isinstance(alloc, mybir.MemoryLocationSet):
            continue
        if not alloc.memorylocations:
            continue
        name = alloc.memorylocations[0].name
        if alloc.kind == "ExternalInput":
            if name != partition_name:
                in_names.append(name)
        elif alloc.kind == "ExternalOutput":
            out_names.append(name)
            shape = tuple(alloc.tensor_shape)
            dtype = mybir.dt.np(alloc.dtype)
            out_avals.append(jax.core.ShapedArray(shape, dtype))
            zero_outs.append(np.zeros(shape, dtype))
    n_params = len(in_names)
    n_outs = len(out_avals)
    all_in_names = list(in_names) + list(out_names)
    if partition_name is not None:
        all_in_names.append(partition_name)
    donate = tuple(range(n_params, n_params + n_outs))

    def _body(*args):
        operands = list(args)
        if partition_name is not None:
            operands.append(bass2jax.partition_id_tensor())
        outs = bass2jax._bass_exec_p.bind(
            *operands,
            out_avals=tuple(out_avals),
            in_names=tuple(all_in_names),
            out_names=tuple(out_names),
            lowering_input_output_aliases=(),
            sim_require_finite=True,
            sim_require_nnan=True,
            nc=nc,
        )
        return tuple(outs)

    devices = jax.devices()[:n_cores]
    mesh = Mesh(np.asarray(devices), ("core",))
    in_specs = (PartitionSpec("core"),) * (n_params + n_outs)
    out_specs = (PartitionSpec("core"),) * n_outs
    jitted = jax.jit(
        shard_map(_body, mesh=mesh, in_specs=in_specs, out_specs=out_specs,
                  check_rep=False),
        donate_argnums=donate, keep_unused=True)

    def run(in_maps):
        per_core = [[np.asarray(m[n]) for n in in_names] for m in in_maps]
        concat_in = [
            np.concatenate([per_core[c][i] for c in range(n_cores)], axis=0)
            for i in range(n_params)]
        concat_zeros = [
            np.zeros((n_cores * z.shape[0], *z.shape[1:]), z.dtype)
            for z in zero_outs]
        outs = jitted(*concat_in, *concat_zeros)
        jax.block_until_ready(outs)
        return [
            {n: np.asarray(outs[i]).reshape(n_cores, *out_avals[i].shape)[c]
             for i, n in enumerate(out_names)}
            for c in range(n_cores)]

    return run


def kernel(**inputs):
    x = np.asarray(inputs["x"])
    h_idx = np.asarray(inputs["h_idx"])
    w_idx = np.asarray(inputs["w_idx"])
    proj_w = np.asarray(inputs["proj_w"], np.float32)
    proj_b = np.asarray(inputs["proj_b"], np.float32)

    if "nc" not in _cache:
        _cache["nc"] = _build()
        _cache["run"] = _make_runner(_cache["nc"], NCORES)

    maps = _in_maps(x, h_idx, w_idx, proj_w, proj_b)
    results = _cache["run"](maps)

    out = np.stack([results[c]["out"].astype(np.float32)
                    for c in range(NCORES)])
    return np.ascontiguousarray(out.reshape(B, N, EMBED))
